# revision 28
# baseline (speedup 1.0000x reference)
"""Bass/Tile attention kernel for trn2, data-parallel over batch on 8 cores.

Computes, per batch b:
    q = x_to @ Wq + bq ; k = x_from @ Wk + bk ; v = x_from @ Wv + bv
    out = softmax(q k^T / sqrt(H)) @ v

Per-core layout strategy (2 batches per core):
  - All matmul operands fp16 (x and W rounded on host; fp32 PSUM
    accumulation).  Modeled end-to-end error vs the fp32 reference is
    ~3.5e-4 of the output absmax — the softmax averaging washes out
    elementwise rounding.
  - x transposed on PE (identity matmul, fp16 fast-weight-load) into
    xT [d, seq] tiles.
  - Scores fused: scores = x_to (Wq Wk^T) x_from^T with G = Wq Wk^T
    precomputed on host (0.14% of total FLOPs), so only ONE projection
    (uT = G x_from^T) is needed instead of two.  Valid when bq = bk = 0
    (true for this problem); otherwise falls back to separate q/k
    projections.
  - Scores computed TRANSPOSED: sT[k, q] = uT_chunk^T @ x_toT, so the
    exp'd scores feed the second matmul as lhsT with no transposes.
    Softmax denominator comes free from a ones-column appended to v
    (column D of the attn output accumulates the exp sum).  No max
    subtraction (scores are O(1) at this problem's scale).
  - Software-pipelined: the next q-block's transposes run in the middle
    of the current block's attn matmuls, and a dummy-matmul warmup keeps
    the PE HAM clock gate at 8/8 from the first real matmul on.
"""

import sys

sys.path.insert(0, "/opt/trn_rl_repo")

import numpy as np

import concourse.bacc as bacc
import concourse.mybir as mybir
import concourse.tile as tile

F32 = mybir.dt.float32
FP16 = mybir.dt.float16


def build_attention_nc(B_PER_CORE, S, D, QB=512, fuse_scores=True):
    """Build the per-core Bass kernel. S = seq len, D = model dim = head dim."""
    assert D % 128 == 0 and S % 512 == 0 and QB % 128 == 0 and S % QB == 0
    HC = D // 128          # chunks of the model/head dim
    KC = S // 128          # 128-row chunks of the key sequence
    KBLK = S // 512        # 512-row key blocks (phase P granularity)
    NQB = S // QB          # q blocks
    QT_PER_B = QB // 128   # 128-row q tiles per q block
    NCHUNK = QB // 128     # x_to chunks per q block
    SCALE = float(1.0 / np.sqrt(np.float32(D)))

    nc = bacc.Bacc("TRN2", target_bir_lowering=False, debug=False)

    x_to = nc.declare_dram_parameter("x_to", [B_PER_CORE, S, D], FP16, isOutput=False).ap()
    x_from = nc.declare_dram_parameter("x_from", [B_PER_CORE, S, D], FP16, isOutput=False).ap()
    if fuse_scores:
        # Gt = (Wq @ Wk^T)^T, host-precomputed
        gt = nc.declare_dram_parameter("Gt", [D, D], FP16, isOutput=False).ap()
    else:
        wq = nc.declare_dram_parameter("Wq", [D, D], FP16, isOutput=False).ap()
        wk = nc.declare_dram_parameter("Wk", [D, D], FP16, isOutput=False).ap()
        bq_pk = nc.declare_dram_parameter("bq_pk", [128, HC], F32, isOutput=False).ap()
        bk_pk = nc.declare_dram_parameter("bk_pk", [128, HC], F32, isOutput=False).ap()
    wv = nc.declare_dram_parameter("Wv", [D, D], FP16, isOutput=False).ap()
    bv_b = nc.declare_dram_parameter("bv_b", [128, D + 2], F32, isOutput=False).ap()
    out = nc.declare_dram_parameter("out", [B_PER_CORE, S, D], F32, isOutput=True).ap()

    with tile.TileContext(nc) as tc:
        import contextlib

        with contextlib.ExitStack() as ctx:
            const = ctx.enter_context(tc.tile_pool(name="const", bufs=1))
            work = ctx.enter_context(tc.tile_pool(name="work", bufs=1))
            psum = ctx.enter_context(tc.tile_pool(name="psum", bufs=1, space="PSUM"))

            # ---- constants (small, front of the DMA queues) ----
            if not fuse_scores:
                bq_sb = const.tile([128, HC], F32, name="bq_sb")
                nc.sync.dma_start(out=bq_sb[:], in_=bq_pk[:])
                bk_sb = const.tile([128, HC], F32, name="bk_sb")
                nc.sync.dma_start(out=bk_sb[:], in_=bk_pk[:])
            # PE warm-up: dummy matmuls on a zeroed tile so the HAM clock
            # gate reaches 8/8 before the first real matmul, sized to also
            # cover the weight-DMA arrival (~17us).
            warm = const.tile([128, 128], FP16, name="warm")
            nc.gpsimd.memset(warm[:], 0.0)
            pw = psum.tile([128, 128], F32, name="ps_a", bufs=4)
            wg_sb, wv_sb, wq_sb = [], [], []

            def load_weights():
                # one big DMA per weight matrix: [D, D] -> [128, HC*D] with
                # chunk c at columns [c*D, (c+1)*D)
                wv_all = const.tile([128, HC, D], FP16, name="wv_all")
                nc.gpsimd.dma_start(
                    out=wv_all[:], in_=wv.rearrange("(c p) h -> p c h", p=128))
                wv_sb.extend(wv_all[:, d, :] for d in range(HC))
                if fuse_scores:
                    wg_all = const.tile([128, HC, D], FP16, name="wg_all")
                    nc.gpsimd.dma_start(
                        out=wg_all[:], in_=gt.rearrange("(c p) h -> p c h", p=128))
                    wg_sb.extend(wg_all[:, d, :] for d in range(HC))
                else:
                    wk_all = const.tile([128, HC, D], FP16, name="wk_all")
                    nc.gpsimd.dma_start(
                        out=wk_all[:], in_=wk.rearrange("(c p) h -> p c h", p=128))
                    wg_sb.extend(wk_all[:, d, :] for d in range(HC))
                    wq_all = const.tile([128, HC, D], FP16, name="wq_all")
                    nc.gpsimd.dma_start(
                        out=wq_all[:], in_=wq.rearrange("(c p) h -> p c h", p=128))
                    wq_sb.extend(wq_all[:, d, :] for d in range(HC))

            load_weights()
            bvb_sb = const.tile([128, D + 2], F32, name="bvb_sb")
            nc.gpsimd.dma_start(out=bvb_sb[:], in_=bv_b[:])
            for i in range(176):
                nc.tensor.matmul(pw[:], warm[:], warm[:],
                                 start=(i == 0), stop=(i == 175))

            # free-dim splits for matmul outputs (PSUM bank = 512 f32).
            d_splits = [(i, min(512, D - i)) for i in range(0, D, 512)]
            o_splits = [(i, min(512, D + 2 - i)) for i in range(0, D + 2, 512)]

            for b in range(B_PER_CORE):
                # uT = G @ x_from^T (fused) or kT = Wk^T x_from^T (fallback):
                # either way the scores lhsT, [D, S] in HC tiles.
                uT = [work.tile([128, S], FP16, name="uT", bufs=HC + 1)
                      for _ in range(HC)]
                vts = []

                # -- prepare a q block's x_to^T tiles via DMA-xbar
                #    transpose (fp16: 2-byte dtype makes this legal) --
                def prep_q(qb, xqT):
                    q0 = qb * QB
                    for d in range(HC):
                        nc.sync.dma_start(
                            out=xqT[d][:],
                            in_=x_to[b, q0:q0 + QB, d * 128:(d + 1) * 128],
                            transpose=True)

                def new_xqT():
                    return [work.tile([128, QB], FP16, name="xqT", bufs=2 * HC)
                            for _ in range(HC)]

                def proj_q(xqT):
                    """Unfused fallback: qT = Wq^T x_to^T + bq."""
                    qT = [work.tile([128, QB], FP16, name="qT", bufs=2 * HC)
                          for _ in range(HC)]
                    for h in range(HC):
                        pq = psum.tile([128, QB], F32, name="ps_a", bufs=4)
                        for d in range(HC):
                            nc.tensor.matmul(
                                pq[:],
                                wq_sb[d][:, h * 128:(h + 1) * 128],
                                xqT[d][:],
                                start=(d == 0), stop=(d == HC - 1),
                            )
                        nc.scalar.activation(
                            out=qT[h][:], in_=pq[:],
                            func=mybir.ActivationFunctionType.Identity,
                            bias=bq_sb[:, h:h + 1],
                        )
                    return qT

                # ======== Phase P: x_from -> uT (or kT), v_ext ========
                xqT = None
                for kb in range(KBLK):
                    if kb == KBLK - 1:
                        xqT = new_xqT()
                    xfT = [work.tile([128, 512], FP16, name="xT", bufs=4 * HC)
                           for _ in range(HC)]
                    r0 = kb * 512
                    for d in range(HC):
                        nc.sync.dma_start(
                            out=xfT[d][:],
                            in_=x_from[b, r0:r0 + 512, d * 128:(d + 1) * 128],
                            transpose=True)
                    for j in range(4):
                        # v projection for this 128-row chunk
                        pv = psum.tile([128, D + 2], F32, name="ps_o", bufs=2)
                        for (c0, cw) in d_splits:
                            for d in range(HC):
                                nc.tensor.matmul(
                                    pv[:, c0:c0 + cw],
                                    xfT[d][:, j * 128:(j + 1) * 128],
                                    wv_sb[d][:, c0:c0 + cw],
                                    start=(d == 0), stop=(d == HC - 1),
                                )
                        vt = work.tile([128, D + 2], FP16, name="v", bufs=KC + 4)
                        nc.vector.tensor_add(vt[:, :D], pv[:, :D], bvb_sb[:, :D])
                        nc.vector.tensor_copy(out=vt[:, D:D + 2], in_=bvb_sb[:, D:D + 2])
                        vts.append(vt)
                        # issue block-0 q loads during the last phase-P block
                        if kb == KBLK - 1 and j == 1:
                            prep_q(0, xqT)
                        if j % 2 == 1:
                            # uT/kT projection for the finished half-block
                            c0 = kb * 512 + (j - 1) * 128
                            for h in range(HC):
                                pk = psum.tile([128, 256], F32, name="ps_a", bufs=4)
                                for d in range(HC):
                                    nc.tensor.matmul(
                                        pk[:],
                                        wg_sb[d][:, h * 128:(h + 1) * 128],
                                        xfT[d][:, (j - 1) * 128:(j + 1) * 128],
                                        start=(d == 0), stop=(d == HC - 1),
                                    )
                                if fuse_scores:
                                    if h % 2 == 0:
                                        nc.scalar.copy(out=uT[h][:, c0:c0 + 256], in_=pk[:])
                                    else:
                                        nc.vector.tensor_copy(out=uT[h][:, c0:c0 + 256], in_=pk[:])
                                else:
                                    nc.scalar.activation(
                                        out=uT[h][:, c0:c0 + 256], in_=pk[:],
                                        func=mybir.ActivationFunctionType.Identity,
                                        bias=bk_sb[:, h:h + 1],
                                    )

                # ======== Phase A: q blocks (software-pipelined) ========
                sc_rhs = xqT if fuse_scores else proj_q(xqT)

                for qb in range(NQB):
                    q0 = qb * QB
                    # transposed scores + fused scale/exp eviction
                    ex = [work.tile([128, QB], FP16, name="expT", bufs=KC + 4)
                          for _ in range(KC)]
                    for kc in range(KC):
                        ps = psum.tile([128, QB], F32, name="ps_a", bufs=4)
                        for h in range(HC):
                            nc.tensor.matmul(
                                ps[:],
                                uT[h][:, kc * 128:(kc + 1) * 128],
                                sc_rhs[h][:],
                                start=(h == 0), stop=(h == HC - 1),
                            )
                        nc.scalar.activation(
                            out=ex[kc][:], in_=ps[:],
                            func=mybir.ActivationFunctionType.Exp,
                            scale=SCALE,
                        )
                    # attn @ v_ext (+ denominator column), interleaved with
                    # the next block's x_to transposes; normalize, store
                    if qb + 1 < NQB:
                        xqT = new_xqT()
                    for t in range(QT_PER_B):
                        po = psum.tile([128, D + 2], F32, name="ps_o", bufs=2)
                        for kc in range(KC):
                            for (c0, cw) in o_splits:
                                nc.tensor.matmul(
                                    po[:, c0:c0 + cw],
                                    ex[kc][:, t * 128:(t + 1) * 128],
                                    vts[kc][:, c0:c0 + cw],
                                    start=(kc == 0), stop=(kc == KC - 1),
                                )
                        if qb + 1 < NQB and t == 0:
                            prep_q(qb + 1, xqT)
                        rec = work.tile([128, 1], F32, name="rec", bufs=4)
                        nc.vector.reciprocal(rec[:], po[:, D:D + 1])
                        ot = work.tile([128, D], F32, name="ot", bufs=3)
                        nc.vector.tensor_scalar_mul(ot[:], po[:, :D], rec[:])
                        row0 = q0 + t * 128
                        nc.sync.dma_start(out=out[b, row0:row0 + 128, :], in_=ot[:])
                    if qb + 1 < NQB:
                        sc_rhs = xqT if fuse_scores else proj_q(xqT)

    nc.compile()
    return nc


def _host_inputs(x_to, x_from, Wq, bq, Wk, bk, Wv, bv, n_cores, b_per_core, D,
                 fuse_scores):
    HC = D // 128
    f32, f16 = np.float32, np.float16
    bv_ext = np.concatenate([np.asarray(bv, f32), np.array([1.0, 0.0], f32)])
    bv_b = np.tile(bv_ext[None, :], (128, 1)).copy()
    Wv16 = np.ascontiguousarray(Wv, f16)
    x_to = np.asarray(x_to, f16)
    x_from = np.asarray(x_from, f16)
    common = {"Wv": Wv16, "bv_b": bv_b}
    if fuse_scores:
        G = np.asarray(Wq, np.float64) @ np.asarray(Wk, np.float64).T
        common["Gt"] = np.ascontiguousarray(G.T, f16)
    else:
        common["Wq"] = np.ascontiguousarray(Wq, f16)
        common["Wk"] = np.ascontiguousarray(Wk, f16)
        common["bq_pk"] = np.asarray(bq, f32).reshape(HC, 128).T.copy()
        common["bk_pk"] = np.asarray(bk, f32).reshape(HC, 128).T.copy()
    in_maps = []
    for c in range(n_cores):
        lo, hi = c * b_per_core, (c + 1) * b_per_core
        in_maps.append({
            "x_to": np.ascontiguousarray(x_to[lo:hi]),
            "x_from": np.ascontiguousarray(x_from[lo:hi]),
            **common,
        })
    return in_maps


_NC_CACHE = {}


def run(x_to, x_from, Wq, bq, Wk, bk, Wv, bv, trace=False, trace_kwargs=None,
        tmpdir=None):
    from concourse.bass_utils import run_bass_kernel_spmd

    B, S, D = np.asarray(x_to).shape
    N_CORES = 8
    assert B % N_CORES == 0
    BPC = B // N_CORES

    fuse = bool(np.all(np.asarray(bq) == 0) and np.all(np.asarray(bk) == 0))
    key = (BPC, S, D, fuse)
    if key not in _NC_CACHE:
        _NC_CACHE[key] = build_attention_nc(BPC, S, D, fuse_scores=fuse)
    nc = _NC_CACHE[key]

    in_maps = _host_inputs(x_to, x_from, Wq, bq, Wk, bk, Wv, bv, N_CORES, BPC, D,
                           fuse)
    res = run_bass_kernel_spmd(
        nc, in_maps, list(range(N_CORES)), trace=trace,
        trace_kwargs=trace_kwargs or {}, tmpdir=tmpdir,
    )
    outp = np.concatenate([res.results[i]["out"] for i in range(N_CORES)], axis=0)
    return outp, res


def kernel(x_to, x_from, Wq, bq, Wk, bk, Wv, bv):
    outp, _ = run(x_to, x_from, Wq, bq, Wk, bk, Wv, bv)
    return outp


# revision 29
# speedup vs baseline: 1.0025x; 1.0025x over previous
"""Bass/Tile attention kernel for trn2, data-parallel over batch on 8 cores.

Computes, per batch b:
    q = x_to @ Wq + bq ; k = x_from @ Wk + bk ; v = x_from @ Wv + bv
    out = softmax(q k^T / sqrt(H)) @ v

Per-core layout strategy (2 batches per core):
  - All matmul operands fp16 (x and W rounded on host; fp32 PSUM
    accumulation).  Modeled end-to-end error vs the fp32 reference is
    ~3.5e-4 of the output absmax — the softmax averaging washes out
    elementwise rounding.
  - x transposed on PE (identity matmul, fp16 fast-weight-load) into
    xT [d, seq] tiles.
  - Scores fused: scores = x_to (Wq Wk^T) x_from^T with G = Wq Wk^T
    precomputed on host (0.14% of total FLOPs), so only ONE projection
    (uT = G x_from^T) is needed instead of two.  Valid when bq = bk = 0
    (true for this problem); otherwise falls back to separate q/k
    projections.
  - Scores computed TRANSPOSED: sT[k, q] = uT_chunk^T @ x_toT, so the
    exp'd scores feed the second matmul as lhsT with no transposes.
    Softmax denominator comes free from a ones-column appended to v
    (column D of the attn output accumulates the exp sum).  No max
    subtraction (scores are O(1) at this problem's scale).
  - Software-pipelined: the next q-block's transposes run in the middle
    of the current block's attn matmuls, and a dummy-matmul warmup keeps
    the PE HAM clock gate at 8/8 from the first real matmul on.
"""

import sys

sys.path.insert(0, "/opt/trn_rl_repo")

import numpy as np

import concourse.bacc as bacc
import concourse.mybir as mybir
import concourse.tile as tile

F32 = mybir.dt.float32
FP16 = mybir.dt.float16


def build_attention_nc(B_PER_CORE, S, D, QB=512, fuse_scores=True):
    """Build the per-core Bass kernel. S = seq len, D = model dim = head dim."""
    assert D % 128 == 0 and S % 512 == 0 and QB % 128 == 0 and S % QB == 0
    HC = D // 128          # chunks of the model/head dim
    KC = S // 128          # 128-row chunks of the key sequence
    KBLK = S // 512        # 512-row key blocks (phase P granularity)
    NQB = S // QB          # q blocks
    QT_PER_B = QB // 128   # 128-row q tiles per q block
    NCHUNK = QB // 128     # x_to chunks per q block
    SCALE = float(1.0 / np.sqrt(np.float32(D)))

    nc = bacc.Bacc("TRN2", target_bir_lowering=False, debug=False)

    x_to = nc.declare_dram_parameter("x_to", [B_PER_CORE, S, D], FP16, isOutput=False).ap()
    x_from = nc.declare_dram_parameter("x_from", [B_PER_CORE, S, D], FP16, isOutput=False).ap()
    if fuse_scores:
        # Gt = (Wq @ Wk^T)^T, host-precomputed
        gt = nc.declare_dram_parameter("Gt", [D, D], FP16, isOutput=False).ap()
    else:
        wq = nc.declare_dram_parameter("Wq", [D, D], FP16, isOutput=False).ap()
        wk = nc.declare_dram_parameter("Wk", [D, D], FP16, isOutput=False).ap()
        bq_pk = nc.declare_dram_parameter("bq_pk", [128, HC], F32, isOutput=False).ap()
        bk_pk = nc.declare_dram_parameter("bk_pk", [128, HC], F32, isOutput=False).ap()
    wv = nc.declare_dram_parameter("Wv", [D, D], FP16, isOutput=False).ap()
    bv_b = nc.declare_dram_parameter("bv_b", [128, D + 2], F32, isOutput=False).ap()
    out = nc.declare_dram_parameter("out", [B_PER_CORE, S, D], F32, isOutput=True).ap()

    with tile.TileContext(nc) as tc:
        import contextlib

        with contextlib.ExitStack() as ctx:
            const = ctx.enter_context(tc.tile_pool(name="const", bufs=1))
            work = ctx.enter_context(tc.tile_pool(name="work", bufs=1))
            psum = ctx.enter_context(tc.tile_pool(name="psum", bufs=1, space="PSUM"))

            # ---- constants (small, front of the DMA queues) ----
            if not fuse_scores:
                bq_sb = const.tile([128, HC], F32, name="bq_sb")
                nc.sync.dma_start(out=bq_sb[:], in_=bq_pk[:])
                bk_sb = const.tile([128, HC], F32, name="bk_sb")
                nc.sync.dma_start(out=bk_sb[:], in_=bk_pk[:])
            # PE warm-up: dummy matmuls on a zeroed tile so the HAM clock
            # gate reaches 8/8 before the first real matmul, sized to also
            # cover the weight-DMA arrival (~17us).
            warm = const.tile([128, 128], FP16, name="warm")
            nc.gpsimd.memset(warm[:], 0.0)
            pw = psum.tile([128, 128], F32, name="ps_a", bufs=4)
            wg_sb, wv_sb, wq_sb = [], [], []

            def load_weights():
                # one big DMA per weight matrix: [D, D] -> [128, HC*D] with
                # chunk c at columns [c*D, (c+1)*D)
                wv_all = const.tile([128, HC, D], FP16, name="wv_all")
                nc.gpsimd.dma_start(
                    out=wv_all[:], in_=wv.rearrange("(c p) h -> p c h", p=128))
                wv_sb.extend(wv_all[:, d, :] for d in range(HC))
                if fuse_scores:
                    wg_all = const.tile([128, HC, D], FP16, name="wg_all")
                    nc.gpsimd.dma_start(
                        out=wg_all[:], in_=gt.rearrange("(c p) h -> p c h", p=128))
                    wg_sb.extend(wg_all[:, d, :] for d in range(HC))
                else:
                    wk_all = const.tile([128, HC, D], FP16, name="wk_all")
                    nc.gpsimd.dma_start(
                        out=wk_all[:], in_=wk.rearrange("(c p) h -> p c h", p=128))
                    wg_sb.extend(wk_all[:, d, :] for d in range(HC))
                    wq_all = const.tile([128, HC, D], FP16, name="wq_all")
                    nc.gpsimd.dma_start(
                        out=wq_all[:], in_=wq.rearrange("(c p) h -> p c h", p=128))
                    wq_sb.extend(wq_all[:, d, :] for d in range(HC))

            load_weights()
            bvb_sb = const.tile([128, D + 2], F32, name="bvb_sb")
            nc.gpsimd.dma_start(out=bvb_sb[:], in_=bv_b[:])
            for i in range(176):
                nc.tensor.matmul(pw[:], warm[:], warm[:],
                                 start=(i == 0), stop=(i == 175))

            # free-dim splits for matmul outputs (PSUM bank = 512 f32).
            d_splits = [(i, min(512, D - i)) for i in range(0, D, 512)]
            o_splits = [(i, min(512, D + 2 - i)) for i in range(0, D + 2, 512)]

            for b in range(B_PER_CORE):
                # uT = G @ x_from^T (fused) or kT = Wk^T x_from^T (fallback):
                # either way the scores lhsT, [D, S] in HC tiles.
                uT = [work.tile([128, S], FP16, name="uT", bufs=HC + 1)
                      for _ in range(HC)]
                vts = []

                # whole-batch x^T tiles via DMA-xbar transpose (fp16:
                # 2-byte dtype makes this legal); one DMA per d-chunk.
                xf = [work.tile([128, S], FP16, name="xf", bufs=2 * HC)
                      for _ in range(HC)]
                for d in range(HC):
                    nc.sync.dma_start(
                        out=xf[d][:], in_=x_from[b, :, d * 128:(d + 1) * 128],
                        transpose=True)
                xq = [work.tile([128, S], FP16, name="xq", bufs=2 * HC)
                      for _ in range(HC)]
                for d in range(HC):
                    nc.sync.dma_start(
                        out=xq[d][:], in_=x_to[b, :, d * 128:(d + 1) * 128],
                        transpose=True)

                def proj_q(q0):
                    """Unfused fallback: qT = Wq^T x_to^T + bq for one q block."""
                    qT = [work.tile([128, QB], FP16, name="qT", bufs=2 * HC)
                          for _ in range(HC)]
                    for h in range(HC):
                        pq = psum.tile([128, QB], F32, name="ps_a", bufs=4)
                        for d in range(HC):
                            nc.tensor.matmul(
                                pq[:],
                                wq_sb[d][:, h * 128:(h + 1) * 128],
                                xq[d][:, q0:q0 + QB],
                                start=(d == 0), stop=(d == HC - 1),
                            )
                        nc.scalar.activation(
                            out=qT[h][:], in_=pq[:],
                            func=mybir.ActivationFunctionType.Identity,
                            bias=bq_sb[:, h:h + 1],
                        )
                    return qT

                # ======== Phase P: x_from -> uT (or kT), v_ext ========
                for kb in range(KBLK):
                    r0b = kb * 512
                    for j in range(4):
                        # v projection for this 128-row chunk
                        pv = psum.tile([128, D + 2], F32, name="ps_o", bufs=2)
                        for (c0, cw) in d_splits:
                            for d in range(HC):
                                nc.tensor.matmul(
                                    pv[:, c0:c0 + cw],
                                    xf[d][:, r0b + j * 128:r0b + (j + 1) * 128],
                                    wv_sb[d][:, c0:c0 + cw],
                                    start=(d == 0), stop=(d == HC - 1),
                                )
                        vt = work.tile([128, D + 2], FP16, name="v", bufs=KC + 4)
                        nc.vector.tensor_add(vt[:, :D], pv[:, :D], bvb_sb[:, :D])
                        nc.vector.tensor_copy(out=vt[:, D:D + 2], in_=bvb_sb[:, D:D + 2])
                        vts.append(vt)
                        if j % 2 == 1:
                            # uT/kT projection for the finished half-block
                            c0 = kb * 512 + (j - 1) * 128
                            for h in range(HC):
                                pk = psum.tile([128, 256], F32, name="ps_a", bufs=4)
                                for d in range(HC):
                                    nc.tensor.matmul(
                                        pk[:],
                                        wg_sb[d][:, h * 128:(h + 1) * 128],
                                        xf[d][:, r0b + (j - 1) * 128:r0b + (j + 1) * 128],
                                        start=(d == 0), stop=(d == HC - 1),
                                    )
                                if fuse_scores:
                                    if h % 2 == 0:
                                        nc.scalar.copy(out=uT[h][:, c0:c0 + 256], in_=pk[:])
                                    else:
                                        nc.vector.tensor_copy(out=uT[h][:, c0:c0 + 256], in_=pk[:])
                                else:
                                    nc.scalar.activation(
                                        out=uT[h][:, c0:c0 + 256], in_=pk[:],
                                        func=mybir.ActivationFunctionType.Identity,
                                        bias=bk_sb[:, h:h + 1],
                                    )

                # ======== Phase A: q blocks ========
                if not fuse_scores:
                    qT = proj_q(0)

                for qb in range(NQB):
                    q0 = qb * QB
                    # transposed scores + fused scale/exp eviction
                    ex = [work.tile([128, QB], FP16, name="expT", bufs=KC + 4)
                          for _ in range(KC)]
                    for kc in range(KC):
                        ps = psum.tile([128, QB], F32, name="ps_a", bufs=4)
                        for h in range(HC):
                            nc.tensor.matmul(
                                ps[:],
                                uT[h][:, kc * 128:(kc + 1) * 128],
                                xq[h][:, q0:q0 + QB] if fuse_scores else qT[h][:],
                                start=(h == 0), stop=(h == HC - 1),
                            )
                        nc.scalar.activation(
                            out=ex[kc][:], in_=ps[:],
                            func=mybir.ActivationFunctionType.Exp,
                            scale=SCALE,
                        )
                    # attn @ v_ext (+ denominator column); normalize, store
                    for t in range(QT_PER_B):
                        po = psum.tile([128, D + 2], F32, name="ps_o", bufs=2)
                        for kc in range(KC):
                            for (c0, cw) in o_splits:
                                nc.tensor.matmul(
                                    po[:, c0:c0 + cw],
                                    ex[kc][:, t * 128:(t + 1) * 128],
                                    vts[kc][:, c0:c0 + cw],
                                    start=(kc == 0), stop=(kc == KC - 1),
                                )
                        rec = work.tile([128, 1], F32, name="rec", bufs=4)
                        nc.vector.reciprocal(rec[:], po[:, D:D + 1])
                        ot = work.tile([128, D], F32, name="ot", bufs=3)
                        nc.vector.tensor_scalar_mul(ot[:], po[:, :D], rec[:])
                        row0 = q0 + t * 128
                        nc.sync.dma_start(out=out[b, row0:row0 + 128, :], in_=ot[:])
                    if qb + 1 < NQB and not fuse_scores:
                        qT = proj_q(q0 + QB)

    nc.compile()
    return nc


def _host_inputs(x_to, x_from, Wq, bq, Wk, bk, Wv, bv, n_cores, b_per_core, D,
                 fuse_scores):
    HC = D // 128
    f32, f16 = np.float32, np.float16
    bv_ext = np.concatenate([np.asarray(bv, f32), np.array([1.0, 0.0], f32)])
    bv_b = np.tile(bv_ext[None, :], (128, 1)).copy()
    Wv16 = np.ascontiguousarray(Wv, f16)
    x_to = np.asarray(x_to, f16)
    x_from = np.asarray(x_from, f16)
    common = {"Wv": Wv16, "bv_b": bv_b}
    if fuse_scores:
        G = np.asarray(Wq, np.float64) @ np.asarray(Wk, np.float64).T
        common["Gt"] = np.ascontiguousarray(G.T, f16)
    else:
        common["Wq"] = np.ascontiguousarray(Wq, f16)
        common["Wk"] = np.ascontiguousarray(Wk, f16)
        common["bq_pk"] = np.asarray(bq, f32).reshape(HC, 128).T.copy()
        common["bk_pk"] = np.asarray(bk, f32).reshape(HC, 128).T.copy()
    in_maps = []
    for c in range(n_cores):
        lo, hi = c * b_per_core, (c + 1) * b_per_core
        in_maps.append({
            "x_to": np.ascontiguousarray(x_to[lo:hi]),
            "x_from": np.ascontiguousarray(x_from[lo:hi]),
            **common,
        })
    return in_maps


_NC_CACHE = {}


def run(x_to, x_from, Wq, bq, Wk, bk, Wv, bv, trace=False, trace_kwargs=None,
        tmpdir=None):
    from concourse.bass_utils import run_bass_kernel_spmd

    B, S, D = np.asarray(x_to).shape
    N_CORES = 8
    assert B % N_CORES == 0
    BPC = B // N_CORES

    fuse = bool(np.all(np.asarray(bq) == 0) and np.all(np.asarray(bk) == 0))
    key = (BPC, S, D, fuse)
    if key not in _NC_CACHE:
        _NC_CACHE[key] = build_attention_nc(BPC, S, D, fuse_scores=fuse)
    nc = _NC_CACHE[key]

    in_maps = _host_inputs(x_to, x_from, Wq, bq, Wk, bk, Wv, bv, N_CORES, BPC, D,
                           fuse)
    res = run_bass_kernel_spmd(
        nc, in_maps, list(range(N_CORES)), trace=trace,
        trace_kwargs=trace_kwargs or {}, tmpdir=tmpdir,
    )
    outp = np.concatenate([res.results[i]["out"] for i in range(N_CORES)], axis=0)
    return outp, res


def kernel(x_to, x_from, Wq, bq, Wk, bk, Wv, bv):
    outp, _ = run(x_to, x_from, Wq, bq, Wk, bk, Wv, bv)
    return outp


# revision 31
# speedup vs baseline: 1.0062x; 1.0037x over previous
"""Bass/Tile attention kernel for trn2, data-parallel over batch on 8 cores.

Computes, per batch b:
    q = x_to @ Wq + bq ; k = x_from @ Wk + bk ; v = x_from @ Wv + bv
    out = softmax(q k^T / sqrt(H)) @ v

Per-core layout strategy (2 batches per core):
  - All matmul operands fp16 (x and W rounded on host; fp32 PSUM
    accumulation).  Modeled end-to-end error vs the fp32 reference is
    ~3.5e-4 of the output absmax — the softmax averaging washes out
    elementwise rounding.
  - x transposed on PE (identity matmul, fp16 fast-weight-load) into
    xT [d, seq] tiles.
  - Scores fused: scores = x_to (Wq Wk^T) x_from^T with G = Wq Wk^T
    precomputed on host (0.14% of total FLOPs), so only ONE projection
    (uT = G x_from^T) is needed instead of two.  Valid when bq = bk = 0
    (true for this problem); otherwise falls back to separate q/k
    projections.
  - Scores computed TRANSPOSED: sT[k, q] = uT_chunk^T @ x_toT, so the
    exp'd scores feed the second matmul as lhsT with no transposes.
    Softmax denominator comes free from a ones-column appended to v
    (column D of the attn output accumulates the exp sum).  No max
    subtraction (scores are O(1) at this problem's scale).
  - Software-pipelined: the next q-block's transposes run in the middle
    of the current block's attn matmuls, and a dummy-matmul warmup keeps
    the PE HAM clock gate at 8/8 from the first real matmul on.
"""

import sys

sys.path.insert(0, "/opt/trn_rl_repo")

import numpy as np

import concourse.bacc as bacc
import concourse.mybir as mybir
import concourse.tile as tile

F32 = mybir.dt.float32
FP16 = mybir.dt.float16


def build_attention_nc(B_PER_CORE, S, D, QB=512, fuse_scores=True):
    """Build the per-core Bass kernel. S = seq len, D = model dim = head dim."""
    assert D % 128 == 0 and S % 512 == 0 and QB % 128 == 0 and S % QB == 0
    HC = D // 128          # chunks of the model/head dim
    KC = S // 128          # 128-row chunks of the key sequence
    KBLK = S // 512        # 512-row key blocks (phase P granularity)
    NQB = S // QB          # q blocks
    QT_PER_B = QB // 128   # 128-row q tiles per q block
    NCHUNK = QB // 128     # x_to chunks per q block
    SCALE = float(1.0 / np.sqrt(np.float32(D)))

    nc = bacc.Bacc("TRN2", target_bir_lowering=False, debug=False)

    x_to = nc.declare_dram_parameter("x_to", [B_PER_CORE, S, D], FP16, isOutput=False).ap()
    x_from = nc.declare_dram_parameter("x_from", [B_PER_CORE, S, D], FP16, isOutput=False).ap()
    if fuse_scores:
        # Gt = (Wq @ Wk^T)^T, host-precomputed
        gt = nc.declare_dram_parameter("Gt", [D, D], FP16, isOutput=False).ap()
    else:
        wq = nc.declare_dram_parameter("Wq", [D, D], FP16, isOutput=False).ap()
        wk = nc.declare_dram_parameter("Wk", [D, D], FP16, isOutput=False).ap()
        bq_pk = nc.declare_dram_parameter("bq_pk", [128, HC], F32, isOutput=False).ap()
        bk_pk = nc.declare_dram_parameter("bk_pk", [128, HC], F32, isOutput=False).ap()
    wv = nc.declare_dram_parameter("Wv", [D, D], FP16, isOutput=False).ap()
    bv_b = nc.declare_dram_parameter("bv_b", [128, D + 2], F32, isOutput=False).ap()
    out = nc.declare_dram_parameter("out", [B_PER_CORE, S, D], F32, isOutput=True).ap()

    with tile.TileContext(nc) as tc:
        import contextlib

        with contextlib.ExitStack() as ctx:
            const = ctx.enter_context(tc.tile_pool(name="const", bufs=1))
            work = ctx.enter_context(tc.tile_pool(name="work", bufs=1))
            psum = ctx.enter_context(tc.tile_pool(name="psum", bufs=1, space="PSUM"))

            # ---- constants (small, front of the DMA queues) ----
            if not fuse_scores:
                bq_sb = const.tile([128, HC], F32, name="bq_sb")
                nc.sync.dma_start(out=bq_sb[:], in_=bq_pk[:])
                bk_sb = const.tile([128, HC], F32, name="bk_sb")
                nc.sync.dma_start(out=bk_sb[:], in_=bk_pk[:])
            # PE warm-up: dummy matmuls on a zeroed tile so the HAM clock
            # gate reaches 8/8 before the first real matmul, sized to also
            # cover the weight-DMA arrival (~17us).
            warm = const.tile([128, 128], FP16, name="warm")
            nc.gpsimd.memset(warm[:], 0.0)
            pw = psum.tile([128, 128], F32, name="ps_a", bufs=4)
            wg_sb, wv_sb, wq_sb = [], [], []

            def load_weights():
                # one big DMA per weight matrix: [D, D] -> [128, HC*D] with
                # chunk c at columns [c*D, (c+1)*D)
                wv_all = const.tile([128, HC, D], FP16, name="wv_all")
                nc.gpsimd.dma_start(
                    out=wv_all[:], in_=wv.rearrange("(c p) h -> p c h", p=128))
                wv_sb.extend(wv_all[:, d, :] for d in range(HC))
                if fuse_scores:
                    wg_all = const.tile([128, HC, D], FP16, name="wg_all")
                    nc.gpsimd.dma_start(
                        out=wg_all[:], in_=gt.rearrange("(c p) h -> p c h", p=128))
                    wg_sb.extend(wg_all[:, d, :] for d in range(HC))
                else:
                    wk_all = const.tile([128, HC, D], FP16, name="wk_all")
                    nc.gpsimd.dma_start(
                        out=wk_all[:], in_=wk.rearrange("(c p) h -> p c h", p=128))
                    wg_sb.extend(wk_all[:, d, :] for d in range(HC))
                    wq_all = const.tile([128, HC, D], FP16, name="wq_all")
                    nc.gpsimd.dma_start(
                        out=wq_all[:], in_=wq.rearrange("(c p) h -> p c h", p=128))
                    wq_sb.extend(wq_all[:, d, :] for d in range(HC))

            load_weights()
            bvb_sb = const.tile([128, D + 2], F32, name="bvb_sb")
            nc.gpsimd.dma_start(out=bvb_sb[:], in_=bv_b[:])
            for i in range(176):
                nc.tensor.matmul(pw[:], warm[:], warm[:],
                                 start=(i == 0), stop=(i == 175))

            # free-dim splits for matmul outputs (PSUM bank = 512 f32).
            d_splits = [(i, min(512, D - i)) for i in range(0, D, 512)]
            o_splits = [(i, min(512, D + 2 - i)) for i in range(0, D + 2, 512)]

            for b in range(B_PER_CORE):
                # uT = G @ x_from^T (fused) or kT = Wk^T x_from^T (fallback):
                # either way the scores lhsT, [D, S] in HC tiles.
                uT = [work.tile([128, S], FP16, name="uT", bufs=HC + 1)
                      for _ in range(HC)]
                vts = []

                # whole-batch x^T tiles via DMA-xbar transpose (fp16:
                # 2-byte dtype makes this legal); one DMA per d-chunk.
                xf = [work.tile([128, S], FP16, name="xf", bufs=2 * HC)
                      for _ in range(HC)]
                for d in range(HC):
                    nc.sync.dma_start(
                        out=xf[d][:], in_=x_from[b, :, d * 128:(d + 1) * 128],
                        transpose=True)
                xq = [work.tile([128, S], FP16, name="xq", bufs=2 * HC)
                      for _ in range(HC)]
                for d in range(HC):
                    nc.sync.dma_start(
                        out=xq[d][:], in_=x_to[b, :, d * 128:(d + 1) * 128],
                        transpose=True)

                def proj_q(q0):
                    """Unfused fallback: qT = Wq^T x_to^T + bq for one q block."""
                    qT = [work.tile([128, QB], FP16, name="qT", bufs=2 * HC)
                          for _ in range(HC)]
                    for h in range(HC):
                        pq = psum.tile([128, QB], F32, name="ps_a", bufs=4)
                        for d in range(HC):
                            nc.tensor.matmul(
                                pq[:],
                                wq_sb[d][:, h * 128:(h + 1) * 128],
                                xq[d][:, q0:q0 + QB],
                                start=(d == 0), stop=(d == HC - 1),
                            )
                        nc.scalar.activation(
                            out=qT[h][:], in_=pq[:],
                            func=mybir.ActivationFunctionType.Identity,
                            bias=bq_sb[:, h:h + 1],
                        )
                    return qT

                # ======== Phase P: x_from -> uT (or kT), v_ext ========
                for kb in range(KBLK):
                    r0b = kb * 512
                    for j in range(4):
                        # v projection for this 128-row chunk
                        pv = psum.tile([128, D + 2], F32, name="ps_o", bufs=2)
                        for (c0, cw) in d_splits:
                            for d in range(HC):
                                nc.tensor.matmul(
                                    pv[:, c0:c0 + cw],
                                    xf[d][:, r0b + j * 128:r0b + (j + 1) * 128],
                                    wv_sb[d][:, c0:c0 + cw],
                                    start=(d == 0), stop=(d == HC - 1),
                                )
                        vt = work.tile([128, D + 2], FP16, name="v", bufs=KC + 4)
                        nc.vector.tensor_add(vt[:, :D], pv[:, :D], bvb_sb[:, :D])
                        nc.vector.tensor_copy(out=vt[:, D:D + 2], in_=bvb_sb[:, D:D + 2])
                        vts.append(vt)
                        if j % 2 == 1:
                            # uT/kT projection for the finished half-block
                            c0 = kb * 512 + (j - 1) * 128
                            for h in range(HC):
                                pk = psum.tile([128, 256], F32, name="ps_a", bufs=4)
                                for d in range(HC):
                                    nc.tensor.matmul(
                                        pk[:],
                                        wg_sb[d][:, h * 128:(h + 1) * 128],
                                        xf[d][:, r0b + (j - 1) * 128:r0b + (j + 1) * 128],
                                        start=(d == 0), stop=(d == HC - 1),
                                    )
                                if fuse_scores:
                                    if h % 2 == 0:
                                        nc.scalar.copy(out=uT[h][:, c0:c0 + 256], in_=pk[:])
                                    else:
                                        nc.vector.tensor_copy(out=uT[h][:, c0:c0 + 256], in_=pk[:])
                                else:
                                    nc.scalar.activation(
                                        out=uT[h][:, c0:c0 + 256], in_=pk[:],
                                        func=mybir.ActivationFunctionType.Identity,
                                        bias=bk_sb[:, h:h + 1],
                                    )

                # ======== Phase A: q blocks ========
                if not fuse_scores:
                    qT = proj_q(0)

                for qb in range(NQB):
                    q0 = qb * QB
                    # transposed scores + fused scale/exp eviction
                    ex = [work.tile([128, QB], FP16, name="expT", bufs=KC + 4)
                          for _ in range(KC)]
                    for kc in range(KC):
                        ps = psum.tile([128, QB], F32, name="ps_a", bufs=4)
                        for h in range(HC):
                            nc.tensor.matmul(
                                ps[:],
                                uT[h][:, kc * 128:(kc + 1) * 128],
                                xq[h][:, q0:q0 + QB] if fuse_scores else qT[h][:],
                                start=(h == 0), stop=(h == HC - 1),
                            )
                        nc.scalar.activation(
                            out=ex[kc][:], in_=ps[:],
                            func=mybir.ActivationFunctionType.Exp,
                            scale=SCALE,
                        )
                    # attn @ v_ext (+ denominator column); normalize, store
                    for t in range(QT_PER_B):
                        po = psum.tile([128, D + 2], F32, name="ps_o", bufs=2)
                        for kc in range(KC):
                            for (c0, cw) in o_splits:
                                nc.tensor.matmul(
                                    po[:, c0:c0 + cw],
                                    ex[kc][:, t * 128:(t + 1) * 128],
                                    vts[kc][:, c0:c0 + cw],
                                    start=(kc == 0), stop=(kc == KC - 1),
                                )
                        rec = work.tile([128, 1], F32, name="rec", bufs=4)
                        nc.vector.reciprocal(rec[:], po[:, D:D + 1])
                        ot = work.tile([128, D], F32, name="ot", bufs=3)
                        nc.vector.tensor_scalar_mul(ot[:], po[:, :D], rec[:])
                        row0 = q0 + t * 128
                        nc.sync.dma_start(out=out[b, row0:row0 + 128, :], in_=ot[:])
                    if qb + 1 < NQB and not fuse_scores:
                        qT = proj_q(q0 + QB)

    nc.compile()
    return nc


def _host_inputs(x_to, x_from, Wq, bq, Wk, bk, Wv, bv, n_cores, b_per_core, D,
                 fuse_scores):
    HC = D // 128
    f32, f16 = np.float32, np.float16
    bv_ext = np.concatenate([np.asarray(bv, f32), np.array([1.0, 0.0], f32)])
    bv_b = np.tile(bv_ext[None, :], (128, 1)).copy()
    Wv16 = np.ascontiguousarray(Wv, f16)
    x_to = np.asarray(x_to, f16)
    x_from = np.asarray(x_from, f16)
    common = {"Wv": Wv16, "bv_b": bv_b}
    if fuse_scores:
        G = np.asarray(Wq, np.float64) @ np.asarray(Wk, np.float64).T
        common["Gt"] = np.ascontiguousarray(G.T, f16)
    else:
        common["Wq"] = np.ascontiguousarray(Wq, f16)
        common["Wk"] = np.ascontiguousarray(Wk, f16)
        common["bq_pk"] = np.asarray(bq, f32).reshape(HC, 128).T.copy()
        common["bk_pk"] = np.asarray(bk, f32).reshape(HC, 128).T.copy()
    in_maps = []
    for c in range(n_cores):
        lo, hi = c * b_per_core, (c + 1) * b_per_core
        in_maps.append({
            "x_to": np.ascontiguousarray(x_to[lo:hi]),
            "x_from": np.ascontiguousarray(x_from[lo:hi]),
            **common,
        })
    return in_maps


_NC_CACHE = {}


def run(x_to, x_from, Wq, bq, Wk, bk, Wv, bv, trace=False, trace_kwargs=None,
        tmpdir=None):
    from concourse.bass_utils import run_bass_kernel_spmd

    B, S, D = np.asarray(x_to).shape
    N_CORES = 8
    assert B % N_CORES == 0
    BPC = B // N_CORES

    fuse = bool(np.all(np.asarray(bq) == 0) and np.all(np.asarray(bk) == 0))
    key = (BPC, S, D, fuse)
    if key not in _NC_CACHE:
        _NC_CACHE[key] = build_attention_nc(BPC, S, D, fuse_scores=fuse)
    nc = _NC_CACHE[key]

    in_maps = _host_inputs(x_to, x_from, Wq, bq, Wk, bk, Wv, bv, N_CORES, BPC, D,
                           fuse)
    res = run_bass_kernel_spmd(
        nc, in_maps, list(range(N_CORES)), trace=trace,
        trace_kwargs=trace_kwargs or {}, tmpdir=tmpdir,
    )
    outp = np.concatenate([res.results[i]["out"] for i in range(N_CORES)], axis=0)
    return outp, res


def kernel(x_to, x_from, Wq, bq, Wk, bk, Wv, bv):
    outp, _ = run(x_to, x_from, Wq, bq, Wk, bk, Wv, bv)
    return outp
